# revision 2
# baseline (speedup 1.0000x reference)
"""Chamfer distance (bidirectional, thresholded) on 8 Trainium2 NeuronCores.

Problem: source_pc/target_pc [2, 16384, 3] fp32 -> [2] fp32.
  dist[b,n,m] = ||src[b,n] - tgt[b,m]||
  out[b] = (mean_n min(min_m dist, T) + mean_m min(min_n dist, T)) / 2

Strategy
--------
Sharding: batch (2) x source-slice (4) over the 8 cores. Each core computes,
for its batch b and its 4096-point source slice:
  * dist1 partial: min_m d2(n, m) for its 4096 n's (complete min over all m)
  * dist2 partial: min_{n in slice} d2(n, m) for all 16384 m's
The host min-reduces dist2 partials across the 4 cores of a batch and applies
sqrt/threshold/mean (cheap: 16K values).

Device kernel: d2 is computed by the TensorEngine via an augmented inner
product of K=30 rows: d2 = x^2 + y^2 - 2xy, with each fp32 operand split into
3 bf16 chunks (exact 24-bit split; cross-chunk product rows give fp32-level
accuracy at bf16 PE speed). K=30 <= 32 enables 4x row-tiling: 4 concurrent
matmuls via tile_position=(32q, 0), each fed from its own 32-partition SBUF
quadrant (inputs quadrant-replicated on host).

Drain: the PE emits -d2 so min becomes max. Each superstep is one
[128, 2048] fp32 PSUM tile. Supersteps are drained in pairs:
  * even ss: ScalarE casts the PSUM tile to f16 in SBUF (line rate).
  * odd ss: VectorE runs tensor_tensor_scan(max, max) with data0 = the odd
    PSUM tile (fp32) and data1 = the even tile's f16 copy, chained across
    pairs via initial=prev_scan[:, -1:]. One scan consumes 4096 values in
    ~2048 DVE cycles - 2x the tensor_reduce drain rate - and its final
    column is the running max of everything so far.
The last scan's final column per output tile is the (negated, squared) min;
a 1-column copy extracts it. f16 rounding of the candidates adds < 5e-4
relative error on d2 (way inside the 2e-2 budget).
"""

import numpy as np
import ml_dtypes

B = 2
N = 16384
M = 16384
CORES = 8
NSLICE = N // 4          # source points per core
T1 = NSLICE // 128       # 32 dist1 output tiles (n on partitions)
T2 = M // 128             # 128 dist2 output tiles (m on partitions)
SS1 = M // 2048           # 8 supersteps (of 4x512 streamed cols) per n-tile
SS2 = NSLICE // 2048      # 2 supersteps per m-tile
KROWS = 30
THRESHOLD = 33.33

# chunk-index pairs (lhs_chunk, rhs_chunk); 0=hi 1=mid 2=lo. (2,2) dropped
# (contributes ~2^-34 relative - far below fp32 rounding of the sum).
_PAIRS = [(0, 0), (0, 1), (1, 0), (0, 2), (2, 0), (1, 1), (1, 2), (2, 1)]

_BF16 = ml_dtypes.bfloat16
NEG_INF16 = -60000.0


def _split3(a):
    """Exact 3-way bf16 split of fp32: a == h + m + l (24-bit mantissa)."""
    h = a.astype(_BF16)
    r = a - h.astype(np.float32)
    m = r.astype(_BF16)
    r2 = r - m.astype(np.float32)
    l = r2.astype(_BF16)
    return h, m, l


def _forms(pts):
    """pts [n,3] fp32 -> (lhs_form, rhs_form), each [KROWS, n] bf16.

    sum_k lhs[k, i] * rhs'[k, j] (for rhs' built from another point set)
    = |p_i|^2 + |q_j|^2 - 2 p_i . q_j  (up to dropped (lo,lo) terms).
    """
    pts = np.ascontiguousarray(pts, dtype=np.float32)
    n = pts.shape[0]
    sq = np.sum(pts * pts, axis=1, dtype=np.float32)
    coord_l = [_split3(np.float32(-2.0) * pts[:, d]) for d in range(3)]
    coord_r = [_split3(pts[:, d]) for d in range(3)]
    sq_c = _split3(sq)
    ones = np.ones(n, dtype=_BF16)
    lhs = np.empty((KROWS, n), dtype=_BF16)
    rhs = np.empty((KROWS, n), dtype=_BF16)
    k = 0
    for d in range(3):
        for (i, j) in _PAIRS:
            lhs[k] = coord_l[d][i]
            rhs[k] = coord_r[d][j]
            k += 1
    for c in range(3):
        lhs[k] = sq_c[c]
        rhs[k] = ones
        k += 1
    for c in range(3):
        lhs[k] = ones
        rhs[k] = sq_c[c]
        k += 1
    assert k == KROWS
    return lhs, rhs


def _quad(a):
    """[KROWS, X] -> [128, X]: replicate into the 4 SBUF quadrants."""
    out = np.zeros((128, a.shape[1]), dtype=a.dtype)
    for q in range(4):
        out[q * 32: q * 32 + KROWS] = a
    return out


def _neg(a):
    """Exact bf16 negation."""
    return (-a.astype(np.float32)).astype(_BF16)


_NC_CACHE = {}


def build_bass(repeat=1):
    """Build (and cache) the single-core Bass/Tile program."""
    if repeat in _NC_CACHE:
        return _NC_CACHE[repeat]

    import concourse.tile as tile
    from concourse import bacc, mybir

    f32 = mybir.dt.float32
    f16 = mybir.dt.float16
    bf16 = mybir.dt.bfloat16
    MAX = mybir.AluOpType.max

    nc = bacc.Bacc(None, target_bir_lowering=False)
    srcLn_d = nc.declare_dram_parameter("srcLn", [128, NSLICE], bf16, isOutput=False)
    srcR_d = nc.declare_dram_parameter("srcR", [128, NSLICE], bf16, isOutput=False)
    tgtLn_d = nc.declare_dram_parameter("tgtLn", [128, M], bf16, isOutput=False)
    tgtR_d = nc.declare_dram_parameter("tgtR", [128, M], bf16, isOutput=False)
    out1_d = nc.declare_dram_parameter("out1", [128, T1], f32, isOutput=True)
    out2_d = nc.declare_dram_parameter("out2", [128, T2], f32, isOutput=True)

    with tile.TileContext(nc) as tc:
        with (
            tc.tile_pool(name="ins", bufs=1) as ins,
            tc.tile_pool(name="psum", bufs=2, space="PSUM") as psum,
            tc.tile_pool(name="casts", bufs=3) as casts,
            tc.tile_pool(name="scans", bufs=3) as scans,
            tc.tile_pool(name="accs", bufs=1) as accs,
            nc.allow_low_precision(reason="f16 drain of -d2 candidates"),
        ):
            s_srcLn = ins.tile([128, NSLICE], bf16, tag="srcLn", name="s_srcLn")
            s_srcR = ins.tile([128, NSLICE], bf16, tag="srcR", name="s_srcR")
            s_tgtLn = ins.tile([128, M], bf16, tag="tgtLn", name="s_tgtLn")
            s_tgtR = ins.tile([128, M], bf16, tag="tgtR", name="s_tgtR")

            nc.sync.dma_start(out=s_srcLn[:, :], in_=srcLn_d[:, :])
            nc.sync.dma_start(out=s_tgtR[:, :], in_=tgtR_d[:, :])
            nc.sync.dma_start(out=s_tgtLn[:, :], in_=tgtLn_d[:, :])
            nc.sync.dma_start(out=s_srcR[:, :], in_=srcR_d[:, :])

            o1 = accs.tile([128, T1], f32, tag="o1", name="o1")
            o2 = accs.tile([128, T2], f32, tag="o2", name="o2")

            def mm_superstep(lhsn, rhs_sb, lt, c0, pa):
                # 4 chunks of -d2 into one [128, 2048] PSUM tile
                for q in range(4):
                    cc = (c0 + q) * 512
                    nc.tensor.matmul(
                        out=pa[:, q * 512:(q + 1) * 512],
                        lhsT=lhsn[q * 32: q * 32 + KROWS, lt],
                        rhs=rhs_sb[q * 32: q * 32 + KROWS, cc:cc + 512],
                        start=True, stop=True,
                        tile_position=(q * 32, 0),
                    )

            def phase(lhsn, rhs_sb, n_t, n_ss, obuf):
                for t in range(n_t):
                    lt = slice(t * 128, (t + 1) * 128)
                    chain = None
                    for half in range(n_ss // 2):
                        pa = psum.tile([128, 2048], f32, name="pa", tag="ps")
                        mm_superstep(lhsn, rhs_sb, lt, (2 * half) * 4, pa)
                        s16 = casts.tile([128, 2048], f16, name="s16", tag="cast")
                        nc.scalar.copy(out=s16, in_=pa)

                        pb = psum.tile([128, 2048], f32, name="pb", tag="ps")
                        mm_superstep(lhsn, rhs_sb, lt, (2 * half + 1) * 4, pb)
                        so = scans.tile([128, 2048], f16, name="so", tag="scan")
                        nc.vector.tensor_tensor_scan(
                            out=so, data0=pb, data1=s16,
                            initial=NEG_INF16 if chain is None else chain,
                            op0=MAX, op1=MAX,
                        )
                        chain = so[:, 2047:2048]
                    # extract the final running max into the per-tile output
                    nc.scalar.copy(out=obuf[:, t:t + 1], in_=chain)

            def whole_body():
                phase(s_srcLn, s_tgtR, T1, SS1, o1)
                phase(s_tgtLn, s_srcR, T2, SS2, o2)
                nc.sync.dma_start(out=out1_d[:, :], in_=o1)
                nc.sync.dma_start(out=out2_d[:, :], in_=o2)

            if repeat == 1:
                whole_body()
            else:
                with tc.For_i(0, repeat, 1):
                    whole_body()

    if not nc.is_finalized():
        nc.finalize()
    _NC_CACHE[repeat] = nc
    return nc


def make_in_maps(source_pc, target_pc):
    """Host-side prep: per-core dicts of quadrant-replicated bf16 forms.

    The 'Ln' arrays are the exact negation of the lhs form, so the PE
    emits -d2.
    """
    source_pc = np.asarray(source_pc, dtype=np.float32)
    target_pc = np.asarray(target_pc, dtype=np.float32)
    tgt_quads = []
    for b in range(B):
        tl, tr = _forms(target_pc[b])
        tgt_quads.append((_quad(_neg(tl)), _quad(tr)))
    in_maps = []
    for c in range(CORES):
        b, qq = divmod(c, 4)
        src_slice = source_pc[b][qq * NSLICE: (qq + 1) * NSLICE]
        sl, sr = _forms(src_slice)
        tln, tr = tgt_quads[b]
        in_maps.append({
            "srcLn": _quad(_neg(sl)),
            "srcR": _quad(sr),
            "tgtLn": tln,
            "tgtR": tr,
        })
    return in_maps


def postprocess(results):
    """Combine per-core outputs into the [B] chamfer distances.

    Device outputs are max(-d2) values, i.e. negated squared mins.
    """
    out = np.zeros(B, dtype=np.float32)
    for b in range(B):
        d1sq = -np.concatenate(
            [results[b * 4 + q]["out1"].T.reshape(-1) for q in range(4)]
        )
        d2sq = -np.max(
            np.stack([results[b * 4 + q]["out2"].T.reshape(-1) for q in range(4)]),
            axis=0,
        )
        d1 = np.minimum(np.sqrt(np.maximum(d1sq, 0.0)), THRESHOLD).mean(
            dtype=np.float64
        )
        d2 = np.minimum(np.sqrt(np.maximum(d2sq, 0.0)), THRESHOLD).mean(
            dtype=np.float64
        )
        out[b] = 0.5 * (d1 + d2)
    return out


def kernel(source_pc, target_pc):
    from concourse.bass_utils import run_bass_kernel_spmd

    nc = build_bass()
    in_maps = make_in_maps(source_pc, target_pc)
    res = run_bass_kernel_spmd(nc, in_maps, list(range(CORES))).results
    return postprocess(res)


# revision 5
# speedup vs baseline: 1.3248x; 1.3248x over previous
"""Chamfer distance (bidirectional, thresholded) on 8 Trainium2 NeuronCores.

Problem: source_pc/target_pc [2, 16384, 3] fp32 -> [2] fp32.
  dist[b,n,m] = ||src[b,n] - tgt[b,m]||
  out[b] = (mean_n min(min_m dist, T) + mean_m min(min_n dist, T)) / 2

Strategy
--------
Sharding: batch (2) x quarter (4) over the 8 cores. Each core computes,
for its batch b:
  * dist1 for its 4096-point source quarter: min over all 16384 targets.
  * dist2 for its 4096-point target quarter: min over all 16384 sources.
Host just concatenates quarters (no cross-core combines) and applies
sqrt/threshold/mean (cheap: 16K values per batch).

Device kernel: d2 is computed by the TensorEngine via an augmented inner
product of K=30 rows: d2 = x^2 + y^2 - 2xy, with each fp32 operand split
into 3 bf16 chunks (exact 24-bit split). K=30 <= 32 enables 4x row-tiling:
4 matmuls per [128, 2048] PSUM tile via tile_position=(32q, 0), each fed
from its own 32-partition SBUF quadrant (inputs quadrant-replicated).

Drain (the bottleneck: 2*16384*4096 = 134M fp32 values/core leave PSUM
through engines that can min-reduce). The PE emits -d2 so min becomes max.
Two superstep kinds, mixed to balance ACT/Pool/DVE:

* Tournament pair (2 ss, 4096 cols): the PE writes paW = -d2 of the four
  EVEN 512-col blocks and pbW = d2(even) - d2(odd) of the four block
  pairs. The odd-block difference comes from one matmul against
  host-precomputed "difference forms" (delta = even - odd split exactly
  into bf16 chunks; the |lhs|^2 rows cancel), so the tournament costs no
  extra PE columns. ScalarE casts paW -> f16 and relu's pbW -> f16 (both
  line-rate PSUM reads), Pool adds them (cpW + relu(pbW) =
  max(-d2_even, -d2_odd), the only elementwise op the toolchain allows on
  Pool), and VectorE max-reduces the 2048 f16 maxes to one partial column.
* Direct (1 ss): VectorE max-reduces the [128, 2048] fp32 PSUM tile.

Partials land in accbuf; one strided reduce per phase produces the
[128, 32] outputs. f16 rounding of candidates adds < 1e-3 relative on d2
(budget is 2e-2).
"""

import numpy as np
import ml_dtypes

B = 2
N = 16384
M = 16384
CORES = 8
NSLICE = N // 4          # points per core slice (both phases)
TILES = NSLICE // 128    # 32 output tiles per phase
SS = M // 2048           # 8 supersteps per tile
KROWS = 30
THRESHOLD = 33.33
NPART = 6                # accbuf partial columns per tile (padded)

# Per-tile drain patterns: (tournament pairs, direct supersteps); 2T+D=8.
# PAT3_PERIOD of every PAT_PERIOD tiles use (3,2); the rest use (2,4).
PAT_PERIOD = 3
PAT3_COUNT = 2           # 2 of every 3 tiles are (3,2), one is (2,4)

_PAIRS = [(0, 0), (0, 1), (1, 0), (0, 2), (2, 0), (1, 1), (1, 2), (2, 1)]
_BF16 = ml_dtypes.bfloat16
NEG_PAD = -60000.0


def _split3(a):
    """Exact 3-way bf16 split of fp32: a == h + m + l (24-bit mantissa)."""
    h = a.astype(_BF16)
    r = a - h.astype(np.float32)
    m = r.astype(_BF16)
    r2 = r - m.astype(np.float32)
    l = r2.astype(_BF16)
    return h, m, l


def _forms(pts):
    """pts [n,3] fp32 -> (lhs_form, rhs_form), each [KROWS, n] bf16."""
    pts = np.ascontiguousarray(pts, dtype=np.float32)
    n = pts.shape[0]
    sq = np.sum(pts * pts, axis=1, dtype=np.float32)
    coord_l = [_split3(np.float32(-2.0) * pts[:, d]) for d in range(3)]
    coord_r = [_split3(pts[:, d]) for d in range(3)]
    sq_c = _split3(sq)
    ones = np.ones(n, dtype=_BF16)
    lhs = np.empty((KROWS, n), dtype=_BF16)
    rhs = np.empty((KROWS, n), dtype=_BF16)
    k = 0
    for d in range(3):
        for (i, j) in _PAIRS:
            lhs[k] = coord_l[d][i]
            rhs[k] = coord_r[d][j]
            k += 1
    for c in range(3):
        lhs[k] = sq_c[c]
        rhs[k] = ones
        k += 1
    for c in range(3):
        lhs[k] = ones
        rhs[k] = sq_c[c]
        k += 1
    assert k == KROWS
    return lhs, rhs


def _forms_diff(pts):
    """Difference rhs-forms: pair even/odd 512-col blocks of pts.

    Returns [KROWS, n//2] bf16 such that matmul(lhs_positive_form, out)
    = d2(even block) - d2(odd block) columnwise. The delta and |.|^2
    difference are computed in fp32 (exact) then 3-way bf16 split, so
    accuracy matches the main forms (~1e-5 absolute).
    """
    pts = np.ascontiguousarray(pts, dtype=np.float32)
    n = pts.shape[0]
    pb = pts.reshape(n // 512, 512, 3)
    ptsA = pb[0::2].reshape(-1, 3)   # even 512-blocks
    ptsB = pb[1::2].reshape(-1, 3)   # odd 512-blocks
    sqA = np.sum(ptsA * ptsA, axis=1, dtype=np.float32)
    sqB = np.sum(ptsB * ptsB, axis=1, dtype=np.float32)
    delta = [_split3(ptsA[:, d] - ptsB[:, d]) for d in range(3)]
    sqd = _split3(sqA - sqB)
    rhs = np.zeros((KROWS, n // 2), dtype=_BF16)
    k = 0
    for d in range(3):
        for (i, j) in _PAIRS:
            rhs[k] = delta[d][j]
            k += 1
    k += 3  # |lhs|^2 rows cancel: stay zero
    for c in range(3):
        rhs[k] = sqd[c]
        k += 1
    assert k == KROWS
    return rhs


def _quad(a):
    """[KROWS, X] -> [128, X]: replicate into the 4 SBUF quadrants."""
    out = np.zeros((128, a.shape[1]), dtype=a.dtype)
    for q in range(4):
        out[q * 32: q * 32 + KROWS] = a
    return out


def _neg(a):
    return (-a.astype(np.float32)).astype(_BF16)


_NC_CACHE = {}


def tile_pattern(t):
    """(n_tourn_pairs, n_direct) for tile index t."""
    if t % PAT_PERIOD < PAT3_COUNT:
        return 3, 2
    return 2, 4


def build_bass(repeat=1):
    if repeat in _NC_CACHE:
        return _NC_CACHE[repeat]

    import concourse.tile as tile
    from concourse import bacc, mybir

    f32 = mybir.dt.float32
    f16 = mybir.dt.float16
    bf16 = mybir.dt.bfloat16
    MAX = mybir.AluOpType.max
    ADD = mybir.AluOpType.add
    AXX = mybir.AxisListType.X
    RELU = mybir.ActivationFunctionType.Relu

    nc = bacc.Bacc(None, target_bir_lowering=False)
    srcLn_d = nc.declare_dram_parameter("srcLn", [128, NSLICE], bf16, isOutput=False)
    srcLp_d = nc.declare_dram_parameter("srcLp", [128, NSLICE], bf16, isOutput=False)
    tgtLn_d = nc.declare_dram_parameter("tgtLn", [128, NSLICE], bf16, isOutput=False)
    tgtLp_d = nc.declare_dram_parameter("tgtLp", [128, NSLICE], bf16, isOutput=False)
    tgtR_d = nc.declare_dram_parameter("tgtR", [128, M], bf16, isOutput=False)
    tgtRd_d = nc.declare_dram_parameter("tgtRd", [128, M // 2], bf16, isOutput=False)
    srcR_d = nc.declare_dram_parameter("srcR", [128, M], bf16, isOutput=False)
    srcRd_d = nc.declare_dram_parameter("srcRd", [128, M // 2], bf16, isOutput=False)
    out1_d = nc.declare_dram_parameter("out1", [128, TILES], f32, isOutput=True)
    out2_d = nc.declare_dram_parameter("out2", [128, TILES], f32, isOutput=True)

    with tile.TileContext(nc) as tc:
        with (
            tc.tile_pool(name="ins", bufs=1) as ins,
            tc.tile_pool(name="psum", bufs=2, space="PSUM") as psum,
            tc.tile_pool(name="cps", bufs=4) as cps,
            tc.tile_pool(name="mxs", bufs=3) as mxs,
            tc.tile_pool(name="accs", bufs=1) as accs,
            nc.allow_low_precision(reason="f16 tournament drain of -d2"),
        ):
            s_srcLn = ins.tile([128, NSLICE], bf16, tag="srcLn", name="s_srcLn")
            s_srcLp = ins.tile([128, NSLICE], bf16, tag="srcLp", name="s_srcLp")
            s_tgtLn = ins.tile([128, NSLICE], bf16, tag="tgtLn", name="s_tgtLn")
            s_tgtLp = ins.tile([128, NSLICE], bf16, tag="tgtLp", name="s_tgtLp")
            s_tgtR = ins.tile([128, M], bf16, tag="tgtR", name="s_tgtR")
            s_tgtRd = ins.tile([128, M // 2], bf16, tag="tgtRd", name="s_tgtRd")
            s_srcR = ins.tile([128, M], bf16, tag="srcR", name="s_srcR")
            s_srcRd = ins.tile([128, M // 2], bf16, tag="srcRd", name="s_srcRd")

            nc.sync.dma_start(out=s_srcLn[:, :], in_=srcLn_d[:, :])
            nc.sync.dma_start(out=s_tgtR[:, :], in_=tgtR_d[:, :])
            nc.sync.dma_start(out=s_srcLp[:, :], in_=srcLp_d[:, :])
            nc.sync.dma_start(out=s_tgtRd[:, :], in_=tgtRd_d[:, :])
            nc.sync.dma_start(out=s_tgtLn[:, :], in_=tgtLn_d[:, :])
            nc.sync.dma_start(out=s_srcR[:, :], in_=srcR_d[:, :])
            nc.sync.dma_start(out=s_tgtLp[:, :], in_=tgtLp_d[:, :])
            nc.sync.dma_start(out=s_srcRd[:, :], in_=srcRd_d[:, :])

            acc1 = accs.tile([128, TILES * NPART], f32, tag="acc1", name="acc1")
            acc2 = accs.tile([128, TILES * NPART], f32, tag="acc2", name="acc2")
            nc.vector.memset(acc1, NEG_PAD)
            nc.vector.memset(acc2, NEG_PAD)
            o1 = accs.tile([128, TILES], f32, tag="o1", name="o1")
            o2 = accs.tile([128, TILES], f32, tag="o2", name="o2")

            def mm4(dst, lhs_sb, lt, rhs_sb, blocks):
                """4 matmuls of 512 cols into dst [128, 2048]."""
                for q in range(4):
                    bb = blocks[q] * 512
                    nc.tensor.matmul(
                        out=dst[:, q * 512:(q + 1) * 512],
                        lhsT=lhs_sb[q * 32: q * 32 + KROWS, lt],
                        rhs=rhs_sb[q * 32: q * 32 + KROWS, bb:bb + 512],
                        start=True, stop=True,
                        tile_position=(q * 32, 0),
                    )

            def tourn_pair(lhsn, lhsp, rhs_sb, rhsd_sb, lt, h):
                # covers rhs blocks 8h..8h+7 (even via pa, odd via diffs)
                paW = psum.tile([128, 2048], f32, name="paW", tag="ps")
                mm4(paW, lhsn, lt, rhs_sb, [8 * h, 8 * h + 2, 8 * h + 4, 8 * h + 6])
                pbW = psum.tile([128, 2048], f32, name="pbW", tag="ps")
                mm4(pbW, lhsp, lt, rhsd_sb, [4 * h, 4 * h + 1, 4 * h + 2, 4 * h + 3])
                # f32 throughout: the add reconstructs max(-d2e, -d2o) by
                # cancellation of O(40)-magnitude operands, so 16-bit
                # rounding before the add corrupts the small winners
                cpW = cps.tile([128, 2048], f32, name="cpW", tag="cp")
                nc.scalar.copy(out=cpW, in_=paW)
                rlW = cps.tile([128, 2048], f32, name="rlW", tag="cp")
                nc.scalar.activation(out=rlW, in_=pbW, func=RELU)
                mxW = mxs.tile([128, 2048], f32, name="mxW", tag="mx")
                nc.gpsimd.tensor_tensor(out=mxW, in0=cpW, in1=rlW, op=ADD)
                return mxW

            def direct_ss(lhsn, rhs_sb, lt, c, accbuf, pc):
                pa = psum.tile([128, 2048], f32, name="pa", tag="ps")
                mm4(pa, lhsn, lt, rhs_sb, [4 * c, 4 * c + 1, 4 * c + 2, 4 * c + 3])
                nc.vector.tensor_reduce(
                    out=accbuf[:, pc:pc + 1], in_=pa, axis=AXX, op=MAX)

            def phase(lhsn, lhsp, rhs_sb, rhsd_sb, accbuf):
                for t in range(TILES):
                    lt = slice(t * 128, (t + 1) * 128)
                    nT, nD = tile_pattern(t)
                    pc = t * NPART
                    # interleave tournament pairs and directs; merge the
                    # tile's mxW leaves with 2x-rate f16 tensor_tensor maxes
                    # and spend only one tensor_reduce on the merged tile
                    mx_run = None
                    nd_done = 0
                    for h in range(nT):
                        mxW = tourn_pair(lhsn, lhsp, rhs_sb, rhsd_sb, lt, h)
                        if mx_run is None:
                            mx_run = mxW
                        else:
                            mrg = mxs.tile([128, 2048], f32, name="mrg", tag="mx")
                            nc.vector.tensor_tensor(
                                out=mrg, in0=mx_run, in1=mxW, op=MAX)
                            mx_run = mrg
                        if nd_done < nD:
                            direct_ss(lhsn, rhs_sb, lt, 2 * nT + nd_done,
                                      accbuf, pc)
                            pc += 1
                            nd_done += 1
                    for c in range(nd_done, nD):
                        direct_ss(lhsn, rhs_sb, lt, 2 * nT + c, accbuf, pc)
                        pc += 1
                    nc.vector.tensor_reduce(
                        out=accbuf[:, pc:pc + 1], in_=mx_run, axis=AXX, op=MAX)

            def whole_body():
                phase(s_srcLn, s_srcLp, s_tgtR, s_tgtRd, acc1)
                phase(s_tgtLn, s_tgtLp, s_srcR, s_srcRd, acc2)
                nc.vector.tensor_reduce(
                    out=o1, in_=acc1.rearrange("p (t s) -> p t s", s=NPART),
                    axis=AXX, op=MAX)
                nc.sync.dma_start(out=out1_d[:, :], in_=o1)
                nc.vector.tensor_reduce(
                    out=o2, in_=acc2.rearrange("p (t s) -> p t s", s=NPART),
                    axis=AXX, op=MAX)
                nc.sync.dma_start(out=out2_d[:, :], in_=o2)

            if repeat == 1:
                whole_body()
            else:
                with tc.For_i(0, repeat, 1):
                    whole_body()

    if not nc.is_finalized():
        nc.finalize()
    _NC_CACHE[repeat] = nc
    return nc


def make_in_maps(source_pc, target_pc):
    source_pc = np.asarray(source_pc, dtype=np.float32)
    target_pc = np.asarray(target_pc, dtype=np.float32)
    per_batch = []
    for b in range(B):
        sl_full, sr_full = _forms(source_pc[b])
        tl_full, tr_full = _forms(target_pc[b])
        srd = _forms_diff(source_pc[b])
        trd = _forms_diff(target_pc[b])
        per_batch.append({
            "srcR": _quad(sr_full), "srcRd": _quad(srd),
            "tgtR": _quad(tr_full), "tgtRd": _quad(trd),
            "sl_full": sl_full, "tl_full": tl_full,
        })
    in_maps = []
    for c in range(CORES):
        b, qq = divmod(c, 4)
        pb = per_batch[b]
        sl = pb["sl_full"][:, qq * NSLICE:(qq + 1) * NSLICE]
        tl = pb["tl_full"][:, qq * NSLICE:(qq + 1) * NSLICE]
        in_maps.append({
            "srcLn": _quad(_neg(sl)),
            "srcLp": _quad(sl),
            "tgtLn": _quad(_neg(tl)),
            "tgtLp": _quad(tl),
            "tgtR": pb["tgtR"],
            "tgtRd": pb["tgtRd"],
            "srcR": pb["srcR"],
            "srcRd": pb["srcRd"],
        })
    return in_maps


def postprocess(results):
    """Combine per-core outputs into the [B] chamfer distances.

    out1/out2 [128, TILES] hold max(-d2) per (partition, tile); quarters
    concatenate directly (each core's mins are complete).
    """
    out = np.zeros(B, dtype=np.float32)
    for b in range(B):
        d1sq = -np.concatenate(
            [results[b * 4 + q]["out1"].T.reshape(-1) for q in range(4)]
        )
        d2sq = -np.concatenate(
            [results[b * 4 + q]["out2"].T.reshape(-1) for q in range(4)]
        )
        d1 = np.minimum(np.sqrt(np.maximum(d1sq, 0.0)), THRESHOLD).mean(
            dtype=np.float64)
        d2 = np.minimum(np.sqrt(np.maximum(d2sq, 0.0)), THRESHOLD).mean(
            dtype=np.float64)
        out[b] = 0.5 * (d1 + d2)
    return out


def kernel(source_pc, target_pc):
    from concourse.bass_utils import run_bass_kernel_spmd

    nc = build_bass()
    in_maps = make_in_maps(source_pc, target_pc)
    res = run_bass_kernel_spmd(nc, in_maps, list(range(CORES))).results
    return postprocess(res)


# revision 7
# speedup vs baseline: 1.6293x; 1.2298x over previous
"""Chamfer distance (bidirectional, thresholded) on 8 Trainium2 NeuronCores.

Problem: source_pc/target_pc [2, 16384, 3] fp32 -> [2] fp32.
  dist[b,n,m] = ||src[b,n] - tgt[b,m]||
  out[b] = (mean_n min(min_m dist, T) + mean_m min(min_n dist, T)) / 2

Strategy
--------
Sharding: batch (2) x quarter (4) over the 8 cores. Each core computes,
for its batch b:
  * dist1 for its 4096-point source quarter: min over all 16384 targets.
  * dist2 for its 4096-point target quarter: min over all 16384 sources.
Host just concatenates quarters (no cross-core combines) and applies
sqrt/threshold/mean (cheap: 16K values per batch).

Device kernel: d2 is computed by the TensorEngine via an augmented inner
product of K=30 rows: d2 = x^2 + y^2 - 2xy, with each fp32 operand split
into 3 bf16 chunks (exact 24-bit split). K=30 <= 32 enables 4x row-tiling:
4 matmuls per [128, 2048] PSUM tile via tile_position=(32q, 0), each fed
from its own 32-partition SBUF quadrant (inputs quadrant-replicated).

Drain (the bottleneck: 2*16384*4096 = 134M fp32 values/core leave PSUM
through engines that can min-reduce). The PE emits -d2 so min becomes max.
Two superstep kinds, mixed to balance ACT/Pool/DVE:

* Tournament pair (2 ss, 4096 cols): the PE writes paW = -d2 of the four
  EVEN 512-col blocks and pbW = d2(even) - d2(odd) of the four block
  pairs. The odd-block difference comes from one matmul against
  host-precomputed "difference forms" (delta = even - odd split exactly
  into bf16 chunks; the |lhs|^2 rows cancel), so the tournament costs no
  extra PE columns. ScalarE casts paW -> f16 and relu's pbW -> f16 (both
  line-rate PSUM reads), Pool adds them (cpW + relu(pbW) =
  max(-d2_even, -d2_odd), the only elementwise op the toolchain allows on
  Pool), and VectorE max-reduces the 2048 f16 maxes to one partial column.
* Direct (1 ss): VectorE max-reduces the [128, 2048] fp32 PSUM tile.

Partials land in accbuf; one strided reduce per phase produces the
[128, 32] outputs. f16 rounding of candidates adds < 1e-3 relative on d2
(budget is 2e-2).
"""

import numpy as np
import ml_dtypes

B = 2
N = 16384
M = 16384
CORES = 8
NSLICE = N // 4          # points per core slice (both phases)
TILES = NSLICE // 128    # 32 output tiles per phase
SS = M // 2048           # 8 supersteps per tile
KROWS = 30
THRESHOLD = 33.33
NPART = 12               # accbuf partial columns per tile (padded)

# Per-tile drain patterns: (tournament pairs, direct supersteps); 2T+D=8.
# PAT3_PERIOD of every PAT_PERIOD tiles use (3,2); the rest use (2,4).
PAT_PERIOD = 3
PAT3_COUNT = 2           # 2 of every 3 tiles are (3,2), one is (2,4)

_PAIRS = [(0, 0), (0, 1), (1, 0), (0, 2), (2, 0), (1, 1), (1, 2), (2, 1)]
_BF16 = ml_dtypes.bfloat16
NEG_PAD = -60000.0


def _split3(a):
    """Exact 3-way bf16 split of fp32: a == h + m + l (24-bit mantissa)."""
    h = a.astype(_BF16)
    r = a - h.astype(np.float32)
    m = r.astype(_BF16)
    r2 = r - m.astype(np.float32)
    l = r2.astype(_BF16)
    return h, m, l


def _forms(pts):
    """pts [n,3] fp32 -> (lhs_form, rhs_form), each [KROWS, n] bf16."""
    pts = np.ascontiguousarray(pts, dtype=np.float32)
    n = pts.shape[0]
    sq = np.sum(pts * pts, axis=1, dtype=np.float32)
    coord_l = [_split3(np.float32(-2.0) * pts[:, d]) for d in range(3)]
    coord_r = [_split3(pts[:, d]) for d in range(3)]
    sq_c = _split3(sq)
    ones = np.ones(n, dtype=_BF16)
    lhs = np.empty((KROWS, n), dtype=_BF16)
    rhs = np.empty((KROWS, n), dtype=_BF16)
    k = 0
    for d in range(3):
        for (i, j) in _PAIRS:
            lhs[k] = coord_l[d][i]
            rhs[k] = coord_r[d][j]
            k += 1
    for c in range(3):
        lhs[k] = sq_c[c]
        rhs[k] = ones
        k += 1
    for c in range(3):
        lhs[k] = ones
        rhs[k] = sq_c[c]
        k += 1
    assert k == KROWS
    return lhs, rhs


def _forms_diff(pts):
    """Difference rhs-forms: pair even/odd 512-col blocks of pts.

    Returns [KROWS, n//2] bf16 such that matmul(lhs_positive_form, out)
    = d2(even block) - d2(odd block) columnwise. The delta and |.|^2
    difference are computed in fp32 (exact) then 3-way bf16 split, so
    accuracy matches the main forms (~1e-5 absolute).
    """
    pts = np.ascontiguousarray(pts, dtype=np.float32)
    n = pts.shape[0]
    pb = pts.reshape(n // 512, 512, 3)
    ptsA = pb[0::2].reshape(-1, 3)   # even 512-blocks
    ptsB = pb[1::2].reshape(-1, 3)   # odd 512-blocks
    sqA = np.sum(ptsA * ptsA, axis=1, dtype=np.float32)
    sqB = np.sum(ptsB * ptsB, axis=1, dtype=np.float32)
    delta = [_split3(ptsA[:, d] - ptsB[:, d]) for d in range(3)]
    sqd = _split3(sqA - sqB)
    rhs = np.zeros((KROWS, n // 2), dtype=_BF16)
    k = 0
    for d in range(3):
        for (i, j) in _PAIRS:
            rhs[k] = delta[d][j]
            k += 1
    k += 3  # |lhs|^2 rows cancel: stay zero
    for c in range(3):
        rhs[k] = sqd[c]
        k += 1
    assert k == KROWS
    return rhs


def _quad(a):
    """[KROWS, X] -> [128, X]: replicate into the 4 SBUF quadrants."""
    out = np.zeros((128, a.shape[1]), dtype=a.dtype)
    for q in range(4):
        out[q * 32: q * 32 + KROWS] = a
    return out


def _neg(a):
    return (-a.astype(np.float32)).astype(_BF16)


_NC_CACHE = {}


def tile_pattern(t):
    """(n_narrow_pairs, n_direct_spans) for tile index t; pairs+spans=8."""
    if t % PAT_PERIOD < PAT3_COUNT:
        return 6, 2
    return 4, 4


def build_bass(repeat=1):
    if repeat in _NC_CACHE:
        return _NC_CACHE[repeat]

    import concourse.tile as tile
    from concourse import bacc, mybir

    f32 = mybir.dt.float32
    f16 = mybir.dt.float16
    bf16 = mybir.dt.bfloat16
    MAX = mybir.AluOpType.max
    ADD = mybir.AluOpType.add
    AXX = mybir.AxisListType.X
    RELU = mybir.ActivationFunctionType.Relu

    nc = bacc.Bacc(None, target_bir_lowering=False)
    srcLn_d = nc.declare_dram_parameter("srcLn", [128, NSLICE], bf16, isOutput=False)
    srcLp_d = nc.declare_dram_parameter("srcLp", [128, NSLICE], bf16, isOutput=False)
    tgtLn_d = nc.declare_dram_parameter("tgtLn", [128, NSLICE], bf16, isOutput=False)
    tgtLp_d = nc.declare_dram_parameter("tgtLp", [128, NSLICE], bf16, isOutput=False)
    tgtR_d = nc.declare_dram_parameter("tgtR", [128, M], bf16, isOutput=False)
    tgtRd_d = nc.declare_dram_parameter("tgtRd", [128, M // 2], bf16, isOutput=False)
    srcR_d = nc.declare_dram_parameter("srcR", [128, M], bf16, isOutput=False)
    srcRd_d = nc.declare_dram_parameter("srcRd", [128, M // 2], bf16, isOutput=False)
    out1_d = nc.declare_dram_parameter("out1", [128, TILES], f32, isOutput=True)
    out2_d = nc.declare_dram_parameter("out2", [128, TILES], f32, isOutput=True)

    with tile.TileContext(nc) as tc:
        with (
            tc.tile_pool(name="ins", bufs=1) as ins,
            tc.tile_pool(name="psum", bufs=4, space="PSUM") as psum,
            tc.tile_pool(name="cps", bufs=4) as cps,
            tc.tile_pool(name="mxs", bufs=3) as mxs,
            tc.tile_pool(name="accs", bufs=1) as accs,
            nc.allow_low_precision(reason="f16 tournament drain of -d2"),
        ):
            s_srcLn = ins.tile([128, NSLICE], bf16, tag="srcLn", name="s_srcLn")
            s_srcLp = ins.tile([128, NSLICE], bf16, tag="srcLp", name="s_srcLp")
            s_tgtLn = ins.tile([128, NSLICE], bf16, tag="tgtLn", name="s_tgtLn")
            s_tgtLp = ins.tile([128, NSLICE], bf16, tag="tgtLp", name="s_tgtLp")
            s_tgtR = ins.tile([128, M], bf16, tag="tgtR", name="s_tgtR")
            s_tgtRd = ins.tile([128, M // 2], bf16, tag="tgtRd", name="s_tgtRd")
            s_srcR = ins.tile([128, M], bf16, tag="srcR", name="s_srcR")
            s_srcRd = ins.tile([128, M // 2], bf16, tag="srcRd", name="s_srcRd")

            nc.sync.dma_start(out=s_srcLn[:, :], in_=srcLn_d[:, :])
            nc.sync.dma_start(out=s_tgtR[:, :], in_=tgtR_d[:, :])
            nc.sync.dma_start(out=s_srcLp[:, :], in_=srcLp_d[:, :])
            nc.sync.dma_start(out=s_tgtRd[:, :], in_=tgtRd_d[:, :])
            nc.sync.dma_start(out=s_tgtLn[:, :], in_=tgtLn_d[:, :])
            nc.sync.dma_start(out=s_srcR[:, :], in_=srcR_d[:, :])
            nc.sync.dma_start(out=s_tgtLp[:, :], in_=tgtLp_d[:, :])
            nc.sync.dma_start(out=s_srcRd[:, :], in_=srcRd_d[:, :])

            acc1 = accs.tile([128, TILES * NPART], f32, tag="acc1", name="acc1")
            acc2 = accs.tile([128, TILES * NPART], f32, tag="acc2", name="acc2")
            nc.vector.memset(acc1, NEG_PAD)
            nc.vector.memset(acc2, NEG_PAD)
            o1 = accs.tile([128, TILES], f32, tag="o1", name="o1")
            o2 = accs.tile([128, TILES], f32, tag="o2", name="o2")

            def mm2(dst, lhs_sb, lt, rhs_sb, blocks, quads):
                """2 matmuls of 512 cols into dst [128, 1024]."""
                for i in range(2):
                    bb = blocks[i] * 512
                    q = quads[i]
                    nc.tensor.matmul(
                        out=dst[:, i * 512:(i + 1) * 512],
                        lhsT=lhs_sb[q * 32: q * 32 + KROWS, lt],
                        rhs=rhs_sb[q * 32: q * 32 + KROWS, bb:bb + 512],
                        start=True, stop=True,
                        tile_position=(q * 32, 0),
                    )

            def tourn_pair_n(lhsn, lhsp, rhs_sb, rhsd_sb, lt, j, qs):
                # span j: orig blocks 4j..4j+3 (even via pa, odd via diffs);
                # f32 until after the add: the add reconstructs
                # max(-d2e, -d2o) by cancellation of O(40) operands, so
                # 16-bit rounding before the add corrupts the small winners
                paN = psum.tile([128, 1024], f32, name="paN", tag="ps")
                mm2(paN, lhsn, lt, rhs_sb, [4 * j, 4 * j + 2], qs[:2])
                pbN = psum.tile([128, 1024], f32, name="pbN", tag="ps")
                mm2(pbN, lhsp, lt, rhsd_sb, [2 * j, 2 * j + 1], qs[2:])
                cpN = cps.tile([128, 1024], f32, name="cpN", tag="cp")
                nc.scalar.copy(out=cpN, in_=paN)
                rlN = cps.tile([128, 1024], f32, name="rlN", tag="cp")
                nc.scalar.activation(out=rlN, in_=pbN, func=RELU)
                mxN = mxs.tile([128, 1024], f16, name="mxN", tag="mx")
                nc.gpsimd.tensor_tensor(out=mxN, in0=cpN, in1=rlN, op=ADD)
                return mxN

            def direct_span(lhsn, rhs_sb, lt, j, qs, accbuf, pc):
                # span j: orig blocks 4j..4j+3 plain, two 1024 tiles
                for i in range(2):
                    pa = psum.tile([128, 1024], f32, name="paD", tag="ps")
                    mm2(pa, lhsn, lt, rhs_sb,
                        [4 * j + 2 * i, 4 * j + 2 * i + 1], qs[2 * i:2 * i + 2])
                    nc.vector.tensor_reduce(
                        out=accbuf[:, pc + i:pc + i + 1], in_=pa, axis=AXX,
                        op=MAX)

            def phase(lhsn, lhsp, rhs_sb, rhsd_sb, accbuf):
                for t in range(TILES):
                    lt = slice(t * 128, (t + 1) * 128)
                    nT, nD = tile_pattern(t)
                    pc = t * NPART
                    # interleave pairs and direct spans; rotate quadrants
                    # so all four PE row-bands stay busy
                    nd_done = 0
                    for h in range(nT):
                        qs = (0, 1, 2, 3) if h % 2 == 0 else (2, 3, 0, 1)
                        mxN = tourn_pair_n(lhsn, lhsp, rhs_sb, rhsd_sb, lt,
                                           h, qs)
                        nc.vector.tensor_reduce(
                            out=accbuf[:, pc:pc + 1], in_=mxN, axis=AXX,
                            op=MAX)
                        pc += 1
                        if nd_done < nD:
                            qs2 = (2, 3, 0, 1) if h % 2 == 0 else (0, 1, 2, 3)
                            direct_span(lhsn, rhs_sb, lt, nT + nd_done, qs2,
                                        accbuf, pc)
                            pc += 2
                            nd_done += 1
                    for c in range(nd_done, nD):
                        qs2 = (0, 1, 2, 3) if c % 2 == 0 else (2, 3, 0, 1)
                        direct_span(lhsn, rhs_sb, lt, nT + c, qs2, accbuf, pc)
                        pc += 2

            def whole_body():
                phase(s_srcLn, s_srcLp, s_tgtR, s_tgtRd, acc1)
                phase(s_tgtLn, s_tgtLp, s_srcR, s_srcRd, acc2)
                nc.vector.tensor_reduce(
                    out=o1, in_=acc1.rearrange("p (t s) -> p t s", s=NPART),
                    axis=AXX, op=MAX)
                nc.sync.dma_start(out=out1_d[:, :], in_=o1)
                nc.vector.tensor_reduce(
                    out=o2, in_=acc2.rearrange("p (t s) -> p t s", s=NPART),
                    axis=AXX, op=MAX)
                nc.sync.dma_start(out=out2_d[:, :], in_=o2)

            if repeat == 1:
                whole_body()
            else:
                with tc.For_i(0, repeat, 1):
                    whole_body()

    if not nc.is_finalized():
        nc.finalize()
    _NC_CACHE[repeat] = nc
    return nc


def make_in_maps(source_pc, target_pc):
    source_pc = np.asarray(source_pc, dtype=np.float32)
    target_pc = np.asarray(target_pc, dtype=np.float32)
    per_batch = []
    for b in range(B):
        sl_full, sr_full = _forms(source_pc[b])
        tl_full, tr_full = _forms(target_pc[b])
        srd = _forms_diff(source_pc[b])
        trd = _forms_diff(target_pc[b])
        per_batch.append({
            "srcR": _quad(sr_full), "srcRd": _quad(srd),
            "tgtR": _quad(tr_full), "tgtRd": _quad(trd),
            "sl_full": sl_full, "tl_full": tl_full,
        })
    in_maps = []
    for c in range(CORES):
        b, qq = divmod(c, 4)
        pb = per_batch[b]
        sl = pb["sl_full"][:, qq * NSLICE:(qq + 1) * NSLICE]
        tl = pb["tl_full"][:, qq * NSLICE:(qq + 1) * NSLICE]
        in_maps.append({
            "srcLn": _quad(_neg(sl)),
            "srcLp": _quad(sl),
            "tgtLn": _quad(_neg(tl)),
            "tgtLp": _quad(tl),
            "tgtR": pb["tgtR"],
            "tgtRd": pb["tgtRd"],
            "srcR": pb["srcR"],
            "srcRd": pb["srcRd"],
        })
    return in_maps


def postprocess(results):
    """Combine per-core outputs into the [B] chamfer distances.

    out1/out2 [128, TILES] hold max(-d2) per (partition, tile); quarters
    concatenate directly (each core's mins are complete).
    """
    out = np.zeros(B, dtype=np.float32)
    for b in range(B):
        d1sq = -np.concatenate(
            [results[b * 4 + q]["out1"].T.reshape(-1) for q in range(4)]
        )
        d2sq = -np.concatenate(
            [results[b * 4 + q]["out2"].T.reshape(-1) for q in range(4)]
        )
        d1 = np.minimum(np.sqrt(np.maximum(d1sq, 0.0)), THRESHOLD).mean(
            dtype=np.float64)
        d2 = np.minimum(np.sqrt(np.maximum(d2sq, 0.0)), THRESHOLD).mean(
            dtype=np.float64)
        out[b] = 0.5 * (d1 + d2)
    return out


def kernel(source_pc, target_pc):
    from concourse.bass_utils import run_bass_kernel_spmd

    nc = build_bass()
    in_maps = make_in_maps(source_pc, target_pc)
    res = run_bass_kernel_spmd(nc, in_maps, list(range(CORES))).results
    return postprocess(res)


# revision 8
# speedup vs baseline: 1.7839x; 1.0949x over previous
"""Chamfer distance (bidirectional, thresholded) on 8 Trainium2 NeuronCores.

Problem: source_pc/target_pc [2, 16384, 3] fp32 -> [2] fp32.
  dist[b,n,m] = ||src[b,n] - tgt[b,m]||
  out[b] = (mean_n min(min_m dist, T) + mean_m min(min_n dist, T)) / 2

Strategy
--------
Sharding: batch (2) x quarter (4) over the 8 cores. Each core computes,
for its batch b:
  * dist1 for its 4096-point source quarter: min over all 16384 targets.
  * dist2 for its 4096-point target quarter: min over all 16384 sources.
Host just concatenates quarters (no cross-core combines) and applies
sqrt/threshold/mean (cheap: 16K values per batch).

Device kernel: d2 is computed by the TensorEngine via an augmented inner
product of K=30 rows: d2 = x^2 + y^2 - 2xy, with each fp32 operand split
into 3 bf16 chunks (exact 24-bit split). K=30 <= 32 enables 4x row-tiling:
4 matmuls per [128, 2048] PSUM tile via tile_position=(32q, 0), each fed
from its own 32-partition SBUF quadrant (inputs quadrant-replicated).

Drain (the bottleneck: 2*16384*4096 = 134M fp32 values/core leave PSUM
through engines that can min-reduce). The PE emits -d2 so min becomes max.
Two superstep kinds, mixed to balance ACT/Pool/DVE:

* Tournament pair (2 ss, 4096 cols): the PE writes paW = -d2 of the four
  EVEN 512-col blocks and pbW = d2(even) - d2(odd) of the four block
  pairs. The odd-block difference comes from one matmul against
  host-precomputed "difference forms" (delta = even - odd split exactly
  into bf16 chunks; the |lhs|^2 rows cancel), so the tournament costs no
  extra PE columns. ScalarE casts paW -> f16 and relu's pbW -> f16 (both
  line-rate PSUM reads), Pool adds them (cpW + relu(pbW) =
  max(-d2_even, -d2_odd), the only elementwise op the toolchain allows on
  Pool), and VectorE max-reduces the 2048 f16 maxes to one partial column.
* Direct (1 ss): VectorE max-reduces the [128, 2048] fp32 PSUM tile.

Partials land in accbuf; one strided reduce per phase produces the
[128, 32] outputs. f16 rounding of candidates adds < 1e-3 relative on d2
(budget is 2e-2).
"""

import numpy as np
import ml_dtypes

B = 2
N = 16384
M = 16384
CORES = 8
NSLICE = N // 4          # points per core slice (both phases)
TILES = NSLICE // 128    # 32 output tiles per phase
SS = M // 2048           # 8 supersteps per tile
KROWS = 30
THRESHOLD = 33.33
NPART = 12               # accbuf partial columns per tile (padded)

# Per-tile drain patterns: (tournament pairs, direct supersteps); 2T+D=8.
# PAT3_PERIOD of every PAT_PERIOD tiles use (3,2); the rest use (2,4).
PAT_PERIOD = 3
PAT3_COUNT = 2           # 2 of every 3 tiles are (3,2), one is (2,4)

_PAIRS = [(0, 0), (0, 1), (1, 0), (0, 2), (2, 0), (1, 1), (1, 2), (2, 1)]
_BF16 = ml_dtypes.bfloat16
NEG_PAD = -60000.0


def _split3(a):
    """Exact 3-way bf16 split of fp32: a == h + m + l (24-bit mantissa)."""
    h = a.astype(_BF16)
    r = a - h.astype(np.float32)
    m = r.astype(_BF16)
    r2 = r - m.astype(np.float32)
    l = r2.astype(_BF16)
    return h, m, l


def _forms(pts):
    """pts [n,3] fp32 -> (lhs_form, rhs_form), each [KROWS, n] bf16."""
    pts = np.ascontiguousarray(pts, dtype=np.float32)
    n = pts.shape[0]
    sq = np.sum(pts * pts, axis=1, dtype=np.float32)
    coord_l = [_split3(np.float32(-2.0) * pts[:, d]) for d in range(3)]
    coord_r = [_split3(pts[:, d]) for d in range(3)]
    sq_c = _split3(sq)
    ones = np.ones(n, dtype=_BF16)
    lhs = np.empty((KROWS, n), dtype=_BF16)
    rhs = np.empty((KROWS, n), dtype=_BF16)
    k = 0
    for d in range(3):
        for (i, j) in _PAIRS:
            lhs[k] = coord_l[d][i]
            rhs[k] = coord_r[d][j]
            k += 1
    for c in range(3):
        lhs[k] = sq_c[c]
        rhs[k] = ones
        k += 1
    for c in range(3):
        lhs[k] = ones
        rhs[k] = sq_c[c]
        k += 1
    assert k == KROWS
    return lhs, rhs


def _forms_diff(pts):
    """Difference rhs-forms: pair even/odd 512-col blocks of pts.

    Returns [KROWS, n//2] bf16 such that matmul(lhs_positive_form, out)
    = d2(even block) - d2(odd block) columnwise. The delta and |.|^2
    difference are computed in fp32 (exact) then 3-way bf16 split, so
    accuracy matches the main forms (~1e-5 absolute).
    """
    pts = np.ascontiguousarray(pts, dtype=np.float32)
    n = pts.shape[0]
    pb = pts.reshape(n // 512, 512, 3)
    ptsA = pb[0::2].reshape(-1, 3)   # even 512-blocks
    ptsB = pb[1::2].reshape(-1, 3)   # odd 512-blocks
    sqA = np.sum(ptsA * ptsA, axis=1, dtype=np.float32)
    sqB = np.sum(ptsB * ptsB, axis=1, dtype=np.float32)
    delta = [_split3(ptsA[:, d] - ptsB[:, d]) for d in range(3)]
    sqd = _split3(sqA - sqB)
    rhs = np.zeros((KROWS, n // 2), dtype=_BF16)
    k = 0
    for d in range(3):
        for (i, j) in _PAIRS:
            rhs[k] = delta[d][j]
            k += 1
    k += 3  # |lhs|^2 rows cancel: stay zero
    for c in range(3):
        rhs[k] = sqd[c]
        k += 1
    assert k == KROWS
    return rhs


def _quad(a):
    """[KROWS, X] -> [128, X]: replicate into the 4 SBUF quadrants."""
    out = np.zeros((128, a.shape[1]), dtype=a.dtype)
    for q in range(4):
        out[q * 32: q * 32 + KROWS] = a
    return out


def _neg(a):
    return (-a.astype(np.float32)).astype(_BF16)


_NC_CACHE = {}


def tile_pattern(t):
    """(n_narrow_pairs, n_direct_spans) for tile index t; pairs+spans=8."""
    if t % PAT_PERIOD < PAT3_COUNT:
        return 6, 2
    return 4, 4


def build_bass(repeat=1):
    if repeat in _NC_CACHE:
        return _NC_CACHE[repeat]

    import concourse.tile as tile
    from concourse import bacc, mybir

    f32 = mybir.dt.float32
    f16 = mybir.dt.float16
    bf16 = mybir.dt.bfloat16
    MAX = mybir.AluOpType.max
    ADD = mybir.AluOpType.add
    AXX = mybir.AxisListType.X
    RELU = mybir.ActivationFunctionType.Relu

    nc = bacc.Bacc(None, target_bir_lowering=False)
    srcLn_d = nc.declare_dram_parameter("srcLn", [128, NSLICE], bf16, isOutput=False)
    srcLp_d = nc.declare_dram_parameter("srcLp", [128, NSLICE], bf16, isOutput=False)
    tgtLn_d = nc.declare_dram_parameter("tgtLn", [128, NSLICE], bf16, isOutput=False)
    tgtLp_d = nc.declare_dram_parameter("tgtLp", [128, NSLICE], bf16, isOutput=False)
    tgtR_d = nc.declare_dram_parameter("tgtR", [128, M], bf16, isOutput=False)
    tgtRd_d = nc.declare_dram_parameter("tgtRd", [128, M // 2], bf16, isOutput=False)
    srcR_d = nc.declare_dram_parameter("srcR", [128, M], bf16, isOutput=False)
    srcRd_d = nc.declare_dram_parameter("srcRd", [128, M // 2], bf16, isOutput=False)
    out1_d = nc.declare_dram_parameter("out1", [128, TILES], f32, isOutput=True)
    out2_d = nc.declare_dram_parameter("out2", [128, TILES], f32, isOutput=True)

    with tile.TileContext(nc) as tc:
        with (
            tc.tile_pool(name="ins", bufs=1) as ins,
            tc.tile_pool(name="psum", bufs=4, space="PSUM") as psum,
            tc.tile_pool(name="cps", bufs=6) as cps,
            tc.tile_pool(name="mxs", bufs=4) as mxs,
            tc.tile_pool(name="accs", bufs=1) as accs,
            nc.allow_low_precision(reason="f16 tournament drain of -d2"),
        ):
            s_srcLn = ins.tile([128, NSLICE], bf16, tag="srcLn", name="s_srcLn")
            s_srcLp = ins.tile([128, NSLICE], bf16, tag="srcLp", name="s_srcLp")
            s_tgtLn = ins.tile([128, NSLICE], bf16, tag="tgtLn", name="s_tgtLn")
            s_tgtLp = ins.tile([128, NSLICE], bf16, tag="tgtLp", name="s_tgtLp")
            s_tgtR = ins.tile([128, M], bf16, tag="tgtR", name="s_tgtR")
            s_tgtRd = ins.tile([128, M // 2], bf16, tag="tgtRd", name="s_tgtRd")
            s_srcR = ins.tile([128, M], bf16, tag="srcR", name="s_srcR")
            s_srcRd = ins.tile([128, M // 2], bf16, tag="srcRd", name="s_srcRd")

            nc.sync.dma_start(out=s_srcLn[:, :], in_=srcLn_d[:, :])
            nc.sync.dma_start(out=s_tgtR[:, :], in_=tgtR_d[:, :])
            nc.sync.dma_start(out=s_srcLp[:, :], in_=srcLp_d[:, :])
            nc.sync.dma_start(out=s_tgtRd[:, :], in_=tgtRd_d[:, :])
            nc.sync.dma_start(out=s_tgtLn[:, :], in_=tgtLn_d[:, :])
            nc.sync.dma_start(out=s_srcR[:, :], in_=srcR_d[:, :])
            nc.sync.dma_start(out=s_tgtLp[:, :], in_=tgtLp_d[:, :])
            nc.sync.dma_start(out=s_srcRd[:, :], in_=srcRd_d[:, :])

            acc1 = accs.tile([128, TILES * NPART], f32, tag="acc1", name="acc1")
            acc2 = accs.tile([128, TILES * NPART], f32, tag="acc2", name="acc2")
            nc.vector.memset(acc1, NEG_PAD)
            nc.vector.memset(acc2, NEG_PAD)
            o1 = accs.tile([128, TILES], f32, tag="o1", name="o1")
            o2 = accs.tile([128, TILES], f32, tag="o2", name="o2")

            def mm2(dst, lhs_sb, lt, rhs_sb, blocks, quads):
                """2 matmuls of 512 cols into dst [128, 1024]."""
                for i in range(2):
                    bb = blocks[i] * 512
                    q = quads[i]
                    nc.tensor.matmul(
                        out=dst[:, i * 512:(i + 1) * 512],
                        lhsT=lhs_sb[q * 32: q * 32 + KROWS, lt],
                        rhs=rhs_sb[q * 32: q * 32 + KROWS, bb:bb + 512],
                        start=True, stop=True,
                        tile_position=(q * 32, 0),
                    )

            def tourn_pair_n(lhsn, lhsp, rhs_sb, rhsd_sb, lt, j, qs):
                # span j: orig blocks 4j..4j+3 (even via pa, odd via diffs);
                # f32 until after the add: the add reconstructs
                # max(-d2e, -d2o) by cancellation of O(40) operands, so
                # 16-bit rounding before the add corrupts the small winners
                paN = psum.tile([128, 1024], f32, name="paN", tag="ps")
                mm2(paN, lhsn, lt, rhs_sb, [4 * j, 4 * j + 2], qs[:2])
                pbN = psum.tile([128, 1024], f32, name="pbN", tag="ps")
                mm2(pbN, lhsp, lt, rhsd_sb, [2 * j, 2 * j + 1], qs[2:])
                cpN = cps.tile([128, 1024], f32, name="cpN", tag="cp")
                nc.scalar.copy(out=cpN, in_=paN)
                rlN = cps.tile([128, 1024], f32, name="rlN", tag="cp")
                nc.scalar.activation(out=rlN, in_=pbN, func=RELU)
                mxN = mxs.tile([128, 1024], f16, name="mxN", tag="mx")
                nc.gpsimd.tensor_tensor(out=mxN, in0=cpN, in1=rlN, op=ADD)
                return mxN

            def direct_span(lhsn, rhs_sb, lt, j, qs, accbuf, pc):
                # span j: orig blocks 4j..4j+3 plain, two 1024 tiles
                for i in range(2):
                    pa = psum.tile([128, 1024], f32, name="paD", tag="ps")
                    mm2(pa, lhsn, lt, rhs_sb,
                        [4 * j + 2 * i, 4 * j + 2 * i + 1], qs[2 * i:2 * i + 2])
                    nc.vector.tensor_reduce(
                        out=accbuf[:, pc + i:pc + i + 1], in_=pa, axis=AXX,
                        op=MAX)

            def phase(lhsn, lhsp, rhs_sb, rhsd_sb, accbuf):
                for t in range(TILES):
                    lt = slice(t * 128, (t + 1) * 128)
                    nT, nD = tile_pattern(t)
                    pc = t * NPART
                    # interleave pairs and direct spans; rotate quadrants
                    # so all four PE row-bands stay busy
                    # spread direct spans evenly among pairs so no engine
                    # sees a long one-sided stretch
                    nd_done = 0
                    stride = max(1, nT // max(nD, 1))
                    for h in range(nT):
                        qs = (0, 1, 2, 3) if h % 2 == 0 else (2, 3, 0, 1)
                        mxN = tourn_pair_n(lhsn, lhsp, rhs_sb, rhsd_sb, lt,
                                           h, qs)
                        nc.vector.tensor_reduce(
                            out=accbuf[:, pc:pc + 1], in_=mxN, axis=AXX,
                            op=MAX)
                        pc += 1
                        if nd_done < nD and h % stride == stride - 1:
                            qs2 = (2, 3, 0, 1) if h % 2 == 0 else (0, 1, 2, 3)
                            direct_span(lhsn, rhs_sb, lt, nT + nd_done, qs2,
                                        accbuf, pc)
                            pc += 2
                            nd_done += 1
                    for c in range(nd_done, nD):
                        qs2 = (0, 1, 2, 3) if c % 2 == 0 else (2, 3, 0, 1)
                        direct_span(lhsn, rhs_sb, lt, nT + c, qs2, accbuf, pc)
                        pc += 2

            def whole_body():
                phase(s_srcLn, s_srcLp, s_tgtR, s_tgtRd, acc1)
                phase(s_tgtLn, s_tgtLp, s_srcR, s_srcRd, acc2)
                nc.vector.tensor_reduce(
                    out=o1, in_=acc1.rearrange("p (t s) -> p t s", s=NPART),
                    axis=AXX, op=MAX)
                nc.sync.dma_start(out=out1_d[:, :], in_=o1)
                nc.vector.tensor_reduce(
                    out=o2, in_=acc2.rearrange("p (t s) -> p t s", s=NPART),
                    axis=AXX, op=MAX)
                nc.sync.dma_start(out=out2_d[:, :], in_=o2)

            if repeat == 1:
                whole_body()
            else:
                with tc.For_i(0, repeat, 1):
                    whole_body()

    if not nc.is_finalized():
        nc.finalize()
    _NC_CACHE[repeat] = nc
    return nc


def make_in_maps(source_pc, target_pc):
    source_pc = np.asarray(source_pc, dtype=np.float32)
    target_pc = np.asarray(target_pc, dtype=np.float32)
    per_batch = []
    for b in range(B):
        sl_full, sr_full = _forms(source_pc[b])
        tl_full, tr_full = _forms(target_pc[b])
        srd = _forms_diff(source_pc[b])
        trd = _forms_diff(target_pc[b])
        per_batch.append({
            "srcR": _quad(sr_full), "srcRd": _quad(srd),
            "tgtR": _quad(tr_full), "tgtRd": _quad(trd),
            "sl_full": sl_full, "tl_full": tl_full,
        })
    in_maps = []
    for c in range(CORES):
        b, qq = divmod(c, 4)
        pb = per_batch[b]
        sl = pb["sl_full"][:, qq * NSLICE:(qq + 1) * NSLICE]
        tl = pb["tl_full"][:, qq * NSLICE:(qq + 1) * NSLICE]
        in_maps.append({
            "srcLn": _quad(_neg(sl)),
            "srcLp": _quad(sl),
            "tgtLn": _quad(_neg(tl)),
            "tgtLp": _quad(tl),
            "tgtR": pb["tgtR"],
            "tgtRd": pb["tgtRd"],
            "srcR": pb["srcR"],
            "srcRd": pb["srcRd"],
        })
    return in_maps


def postprocess(results):
    """Combine per-core outputs into the [B] chamfer distances.

    out1/out2 [128, TILES] hold max(-d2) per (partition, tile); quarters
    concatenate directly (each core's mins are complete).
    """
    out = np.zeros(B, dtype=np.float32)
    for b in range(B):
        d1sq = -np.concatenate(
            [results[b * 4 + q]["out1"].T.reshape(-1) for q in range(4)]
        )
        d2sq = -np.concatenate(
            [results[b * 4 + q]["out2"].T.reshape(-1) for q in range(4)]
        )
        d1 = np.minimum(np.sqrt(np.maximum(d1sq, 0.0)), THRESHOLD).mean(
            dtype=np.float64)
        d2 = np.minimum(np.sqrt(np.maximum(d2sq, 0.0)), THRESHOLD).mean(
            dtype=np.float64)
        out[b] = 0.5 * (d1 + d2)
    return out


def kernel(source_pc, target_pc):
    from concourse.bass_utils import run_bass_kernel_spmd

    nc = build_bass()
    in_maps = make_in_maps(source_pc, target_pc)
    res = run_bass_kernel_spmd(nc, in_maps, list(range(CORES))).results
    return postprocess(res)
